# revision 1
# baseline (speedup 1.0000x reference)
"""Trainium2 Bass kernel for nn_CaptionModel (GRU caption decoder).

Math (per reference):
  h0 = feat @ w_hp + b_hp                      [B, H]
  x0 = embed[SOS]  (broadcast over batch)
  for t in 1..200:  h_t = GRUCell(x_{t-1}, h_{t-1})  with x_t = h_t
  out[b, v, t] = (h_t @ w_proj + b_proj)[b, v]

Key algebra: for t >= 2 the GRU input x equals h, so the r/z gates fold into
a combined weight W'_r = w_ih_r + w_hh_r (same for z); the n gate keeps
w_ih_n / w_hh_n separate (r multiplies only the h-side):
  pre = h @ W'.T,  W' = [W'_r; W'_z; w_ih_n; w_hh_n]   [2048, 512]
  r = sig(pre_r), z = sig(pre_z), n = tanh(pre_in + r * pre_hn)
  h' = n + z*(h - n) = (1-z)*n + z*h
Step 1 input x0 is batch-constant: g0 = w_ih @ embed[SOS] + b_ih folds into
full [H]-shaped activation bias tiles.

Device layout (per core, batch slice Bc=64, pure data parallel over 8 cores;
default variant "h2"): everything transposed, hT [H=512 -> 4 partition-chunks
of 128, Bc=64 free]. Hardware facts that shaped the design (measured via the
differential wall-clock harness in time_harness.py -- the sim's cost model
does not include PE weight-load time and badly mispredicts here):
  * A [128x128] bf16 matmul costs ~27-32 ns regardless of N<=64: the PE is
    WEIGHT-LOAD bound (~2 rows/cycle), so the 64-matmul gate stream is
    ~2.06 us/step and splitting the batch into groups doubles it (each
    group reloads the same weights). One batch group with N=64 is optimal.
  * fp8 DoubleRow matmuls load ~4x slower per instruction -- no win.
  * GPSIMD (Pool) cannot access PSUM and runs tensor ops at 0.42
    efficiency; the Act engine charges ~185 ns of SBUF access per op;
    every dependency edge costs ~100-270 ns (sem + pipeline drain).
The recurrence is therefore latency-bound: P = h'->PE edge + gate stream +
PSUM drain + the serial tail hanging off the LAST gate arrival. The kernel
minimizes that tail:
  PE:   gate order r, z, hn, in (the in-gate's tail is the shortest), then
        the previous step's projection (reads the double-buffered old h, so
        it never blocks the chain).
  PSUM: per-gate banks, with hn split into H-half banks and in split into
        [chunks 0-1][2][3] banks -- dependency tracking is tile-granular,
        so each tail piece waits only on its own matmuls.
  Act:  sig_r, sig_z (full width, hidden under the stream), then tanh in
        three pieces [0:2],[2],[3].
  DVE:  t1 = r*hn (H-halves), t2 = t1 + in, v = (1-z)*n, h' = v + q in
        [0:2],[2],[3] pieces so the last piece's chain after the final
        matmul is one 64-wide op per stage (edges dominate; all-SBUF bf16
        packed ops hit the 4x DVE mode).
  Pool: q = z*h, u = 1-z, both off-chain (SBUF only).
Measured ~2.7 us/step in the looped timing harness vs ~5.5 us/step for the
previous-session baseline measured the same way (~5.15 us/step true), i.e.
about 2x; estimated full-kernel device time ~510-550 us.
"""

import numpy as np
from contextlib import ExitStack

import concourse.bass as bass
import concourse.bacc as bacc
import concourse.mybir as mybir
import concourse.tile as tile
from concourse.bass_utils import run_bass_kernel_spmd

B, FEAT, H, V = 512, 2048, 512, 100
STEPS = 200
SOS = 0
NCORES = 8
Bc = B // NCORES           # 64 batch rows per core
NG = 2                     # ping-pong groups per core
Bg = Bc // NG              # 32 batch rows per group
KC = H // 128              # 4 contraction chunks over H
KF = FEAT // 128           # 16 contraction chunks over FEAT
F32 = mybir.dt.float32
BF16 = mybir.dt.bfloat16
AF = mybir.ActivationFunctionType
OP = mybir.AluOpType

BF16_NP = mybir.dt.np(BF16)
FP8 = mybir.dt.float8e4
FP8_NP = mybir.dt.np(FP8)
KP = KC // 2               # DoubleRow k-pairs (K=256 per instruction)
FP8_WSCALE = 64.0          # fp8 gate weights are stored x64

LAST_RESULTS = None        # test harness introspection (profile/timing)

_PROGRAM_CACHE = {}

# gate index inside wT columns and the PSUM gate bank: r, z, in, hn
GI_R, GI_Z, GI_IN, GI_HN = 0, 1, 2, 3


def _build(nc_biases, steps=STEPS, reps=1, mode="full", variant="split",
           ngroups=NG, out_steps=None):
    """Build the Bass program. nc_biases: frozenset of nonzero bias groups in
    {"rz", "hn", "in", "hp", "proj"} (grading inputs are all-zero biases, so
    the hot path emits no bias work beyond the step-1 g0 fold).
    variant: "split" = per-gate sigmoids; "merged" = one sigmoid over [r|z]."""
    merged = (variant == "merged")
    fp8 = (variant == "fp8")
    h2 = (variant == "h2")
    assert not (h2 and ngroups != 1)
    out_steps = out_steps or steps
    ng, bg = ngroups, Bc // ngroups
    nc = bacc.Bacc(debug=False)

    wT_d = nc.dram_tensor("wT", [KC, 128, 4 * H], BF16, kind="ExternalInput")
    wf8_d = nc.dram_tensor("wf8", [KP, 128, 2, 4 * H], FP8,
                           kind="ExternalInput")
    whhT_d = nc.dram_tensor("whhT", [KC, 128, 3 * H], BF16, kind="ExternalInput")
    whpT_d = nc.dram_tensor("whpT", [KF, 128, H], BF16, kind="ExternalInput")
    featT_d = nc.dram_tensor("featT", [KF, 128, Bc], BF16, kind="ExternalInput")
    wproj_d = nc.dram_tensor("wproj", [KC, 128, V], BF16, kind="ExternalInput")
    # Step-1 activation biases (g0 folded; always present): [128, KC, bg],
    # chunk-major, broadcast over the bg batch columns of one group.
    b1r_d = nc.dram_tensor("b1r", [128, KC, bg], F32, kind="ExternalInput")
    b1z_d = nc.dram_tensor("b1z", [128, KC, bg], F32, kind="ExternalInput")
    b1n_d = nc.dram_tensor("b1n", [128, KC, bg], F32, kind="ExternalInput")
    has_rz = "rz" in nc_biases
    has_hn = "hn" in nc_biases
    has_in = "in" in nc_biases
    has_hp = "hp" in nc_biases
    has_proj = "proj" in nc_biases
    optd = {}
    if has_rz:
        optd["brz"] = nc.dram_tensor("brz", [128, 2, KC, bg], F32,
                                     kind="ExternalInput")
    if has_hn:
        optd["bhn"] = nc.dram_tensor("bhn", [128, KC, bg], F32,
                                     kind="ExternalInput")
    if has_in:
        optd["bin"] = nc.dram_tensor("bin", [128, KC, bg], F32,
                                     kind="ExternalInput")
    if has_hp:
        bhp_d = nc.dram_tensor("bhp", [128, KC], F32, kind="ExternalInput")
    if has_proj:
        bproj_d = nc.dram_tensor("bproj", [Bc, V], F32, kind="ExternalInput")
    out_d = nc.dram_tensor("out", [Bc, V, out_steps], F32,
                           kind="ExternalOutput")

    with tile.TileContext(nc) as tc, ExitStack() as ctx:
        const = ctx.enter_context(tc.tile_pool(name="const", bufs=1))
        hpool = ctx.enter_context(tc.tile_pool(name="h", bufs=2))
        ew = ctx.enter_context(tc.tile_pool(name="ew", bufs=3))
        psum = ctx.enter_context(
            tc.tile_pool(name="psum", bufs=1, space=bass.MemorySpace.PSUM)
        )

        # ---- constants into SBUF ----
        wT = const.tile([128, KC, 4 * H], BF16)
        wf8 = None
        if fp8:
            wf8 = const.tile([128, KP, 2, 4 * H], FP8, name="wf8")
            for kp in range(KP):
                nc.sync.dma_start(wf8[:, kp], wf8_d[kp])
        whhT = const.tile([128, KC, 3 * H], BF16)
        whpT = const.tile([128, KF, H], BF16)
        featT = const.tile([128, KF, Bc], BF16)
        wproj = const.tile([128, KC, V], BF16)
        for k in range(KC):
            if not fp8:
                nc.sync.dma_start(wT[:, k, :], wT_d[k])
            nc.sync.dma_start(whhT[:, k, :], whhT_d[k])
            nc.sync.dma_start(wproj[:, k, :], wproj_d[k])
        for k in range(KF):
            nc.sync.dma_start(whpT[:, k, :], whpT_d[k])
            nc.sync.dma_start(featT[:, k, :], featT_d[k])
        b1r = const.tile([128, KC, bg], F32)
        b1z = const.tile([128, KC, bg], F32)
        b1n = const.tile([128, KC, bg], F32)
        nc.sync.dma_start(b1r[:], b1r_d[:])
        nc.sync.dma_start(b1z[:], b1z_d[:])
        nc.sync.dma_start(b1n[:], b1n_d[:])
        opt = {}
        for name, d in optd.items():
            t = const.tile(list(d.shape), F32)
            nc.sync.dma_start(t[:], d[:])
            opt[name] = t
        if has_hp:
            bhp = const.tile([128, KC], F32)
            nc.sync.dma_start(bhp[:], bhp_d[:])
        if has_proj:
            bproj = const.tile([Bc, V], F32)
            nc.sync.dma_start(bproj[:], bproj_d[:])

        logits = const.tile([Bc, V, out_steps], F32)

        # ---- PSUM gate tiles (bank-granular allocator: 8 banks total).
        # Dependency tracking is tile-granular, so tiles are packed to make
        # each consumer's wait match its true position in the chain.
        # Single-buffered: every reader finishes well before the next
        # step's matmuls land.
        if h2:
            # single group, eight banks: [r], [z] full; hn split into
            # per-H-half banks; in split into [chunks 0-1], [2], [3] so the
            # tail's last pieces wait only their own matmuls (dependency
            # tracking is tile-granular). Arrival order r, z, hn, in.
            rt1 = psum.tile([128, KC, Bc], F32, tag="rt1", bufs=1, name="rt1")
            zt1 = psum.tile([128, KC, Bc], F32, tag="zt1", bufs=1, name="zt1")
            hnh = [psum.tile([128, 2, Bc], F32, tag=f"hnh{h}", bufs=1,
                             name=f"hnh{h}") for h in range(2)]
            in01 = psum.tile([128, 2, Bc], F32, tag="in01", bufs=1,
                             name="in01")
            in2 = psum.tile([128, Bc], F32, tag="in2", bufs=1, name="in2")
            in3 = psum.tile([128, Bc], F32, tag="in3", bufs=1, name="in3")
            rt, hnt, zit = [rt1], None, None
        elif merged:
            # per group: [r|z] (sig_rz), [hn] (T1), [in] (T2);
            # arrival order r, z, hn, in
            rt = [psum.tile([128, 2, KC, bg], F32, tag=f"rt{g}", bufs=1,
                            name=f"rt{g}") for g in range(ng)]
            hnt = [psum.tile([128, KC, bg], F32, tag=f"hnt{g}", bufs=1,
                             name=f"hnt{g}") for g in range(ng)]
            zit = [psum.tile([128, KC, bg], F32, tag=f"zit{g}", bufs=1,
                             name=f"zit{g}") for g in range(ng)]
        else:
            # per group: [r] (sig_r), [hn] (T1), [in|z] (T2 / sig_z);
            # arrival order r, hn, in, z; zit[:, 0] = in, zit[:, 1] = z
            rt = [psum.tile([128, KC, bg], F32, tag=f"rt{g}", bufs=1,
                            name=f"rt{g}") for g in range(ng)]
            hnt = [psum.tile([128, KC, bg], F32, tag=f"hnt{g}", bufs=1,
                             name=f"hnt{g}") for g in range(ng)]
            zit = [psum.tile([128, 2, KC, bg], F32, tag=f"zit{g}", bufs=1,
                             name=f"zit{g}") for g in range(ng)]

        # ---- h0 = feat @ w_hp (+ b_hp), accumulated into the r-gate banks
        hbf_cur = hpool.tile([128, KC, Bc], BF16, tag="hbf", bufs=2)
        hq_cur = (hpool.tile([128, KC, Bc], FP8, tag="hq", bufs=2,
                             name="hq") if fp8 else None)
        for g in range(ng):
            h0t = rt[g][:, 0] if merged else rt[g][:]
            for m in range(KC):
                for k in range(KF):
                    nc.tensor.matmul(
                        h0t[:, m, :],
                        whpT[:, k, m * 128:(m + 1) * 128],
                        featT[:, k, g * bg:(g + 1) * bg],
                        start=(k == 0), stop=(k == KF - 1),
                    )
            hslice = hbf_cur[:, :, g * bg:(g + 1) * bg]
            if has_hp:
                for m in range(KC):
                    nc.vector.tensor_scalar_add(hslice[:, m, :], h0t[:, m, :],
                                                bhp[:, m:m + 1])
            else:
                nc.vector.tensor_copy(hslice, h0t)
            if fp8:
                nc.vector.tensor_copy(
                    hq_cur[:, :, g * bg:(g + 1) * bg], h0t)

        # ---- recurrence ----
        def emit_h2_mms(t, rhs, ksplit=True):
            """Gate matmuls, arrival order r, z, hn(h0,h1), in(h0,h1).
            With ksplit, each tile's k-accumulation is split into a k01
            phase (reads only h chunks 0-1, so it runs during the previous
            step's tail while the PE would otherwise idle) and a k23 phase
            (after the h upper half lands)."""
            first = (t == 1)
            wsrc = whhT if first else wT
            m0_hn = 2 * H if first else 3 * H
            tiles = []   # (dst, weight column base)
            for dstt, m0 in ((rt1, 0), (zt1, H)):
                for ci in range(KC):
                    tiles.append((dstt[:, ci, :], m0 + ci * 128))
            for hf in range(2):
                for cj in range(2):
                    ci = 2 * hf + cj
                    tiles.append((hnh[hf][:, cj, :], m0_hn + ci * 128))
            if not first:
                for cj in range(2):
                    tiles.append((in01[:, cj, :], 2 * H + cj * 128))
                tiles.append((in2[:], 2 * H + 2 * 128))
                tiles.append((in3[:], 2 * H + 3 * 128))
            # tile-major emission measured fastest (k-outer phasing and a
            # k01/k23 split both regress: interleaved PSUM accumulation
            # groups appear to break the PE's weight-load pipelining)
            phases = ((0, 1, 2, 3),)
            for ks in phases:
                for dst, c0 in tiles:
                    for k in ks:
                        nc.tensor.matmul(
                            dst, wsrc[:, k, c0: c0 + 128], rhs[:, k, :],
                            start=(k == 0), stop=(k == KC - 1))

        def emit_h2_tail(t, hbf_prev, hbf_next):
            first = (t == 1)
            r2 = ew.tile([128, KC, Bc], BF16, tag="r2h")
            z2 = ew.tile([128, KC, Bc], BF16, tag="z2h")
            q2 = ew.tile([128, KC, Bc], BF16, tag="q2h")
            u2 = ew.tile([128, KC, Bc], BF16, tag="u2h")
            t1h = [ew.tile([128, 2, Bc], BF16, tag=f"t1h{h}", name=f"t1h{h}")
                   for h in range(2)]
            segw = (2, 1, 1)
            t2h = [ew.tile([128, segw[s], Bc], BF16, tag=f"t2h{s}",
                           name=f"t2h{s}") for s in range(3)]
            n2h = [ew.tile([128, segw[s], Bc], BF16, tag=f"n2h{s}",
                           name=f"n2h{s}") for s in range(3)]
            v2h = [ew.tile([128, segw[s], Bc], BF16, tag=f"v2h{s}",
                           name=f"v2h{s}") for s in range(3)]

            # sigmoids (Act), full width
            if first or has_rz:
                badd = ew.tile([128, 2, KC, Bc], F32, tag="baddh")
                br = b1r[:] if first else opt["brz"][:, 0]
                bz = b1z[:] if first else opt["brz"][:, 1]
                nc.vector.tensor_add(badd[:, 0], rt1[:], br)
                nc.vector.tensor_add(badd[:, 1], zt1[:], bz)
                nc.scalar.activation(r2[:], badd[:, 0], AF.Sigmoid)
                nc.scalar.activation(z2[:], badd[:, 1], AF.Sigmoid)
            else:
                nc.scalar.activation(r2[:], rt1[:], AF.Sigmoid)
                nc.scalar.activation(z2[:], zt1[:], AF.Sigmoid)

            # off-chain (Pool): q = z*h, u = 1-z
            nc.gpsimd.tensor_mul(q2[:], z2[:], hbf_prev[:])
            nc.gpsimd.tensor_scalar(u2[:], z2[:], -1.0, 1.0, OP.mult, OP.add)

            # t1 at halves (hn banks); t2/tanh/v/h' at [0:2], [2], [3]
            for hf in range(2):
                sl = slice(2 * hf, 2 * hf + 2)
                if has_hn:
                    hnb = ew.tile([128, 2, Bc], F32, tag=f"hnbh{hf}",
                                  name=f"hnbh{hf}")
                    nc.vector.tensor_add(hnb[:], hnh[hf][:],
                                         opt["bhn"][:, sl, :])
                    nc.vector.tensor_mul(t1h[hf][:], r2[:, sl, :], hnb[:])
                else:
                    nc.vector.tensor_mul(t1h[hf][:], r2[:, sl, :],
                                         hnh[hf][:])
            segs = (
                (slice(0, 2), in01[:], t1h[0][:]),
                (slice(2, 3), in2[:, None, :], t1h[1][:, 0:1, :]),
                (slice(3, 4), in3[:, None, :], t1h[1][:, 1:2, :]),
            )
            for si, (sl, inap, t1ap) in enumerate(segs):
                if first:
                    nc.vector.tensor_add(t2h[si][:], t1ap, b1n[:, sl, :])
                else:
                    nc.vector.tensor_add(t2h[si][:], t1ap, inap)
                    if has_in:
                        nc.vector.tensor_add(t2h[si][:], t2h[si][:],
                                             opt["bin"][:, sl, :])
                nc.scalar.activation(n2h[si][:], t2h[si][:], AF.Tanh)
            for si, (sl, inap, t1ap) in enumerate(segs):
                nc.vector.tensor_mul(v2h[si][:], u2[:, sl, :], n2h[si][:])
                nc.vector.tensor_add(hbf_next[:, sl, :], v2h[si][:],
                                     q2[:, sl, :])

        def emit_group_mms(t, g, rhs, rhs8=None):
            first = (t == 1)
            if fp8 and not first:
                # DoubleRow fp8: K=256 per instruction via k-chunk pairs
                gates = ((rt[g][:], 0), (hnt[g][:], 3 * H),
                         (zit[g][:, 0], 2 * H), (zit[g][:, 1], H))
                for dstt, m0 in gates:
                    for ci in range(KC):
                        dst = dstt[:, ci, :]
                        for kp in range(KP):
                            nc.tensor.matmul(
                                dst,
                                wf8[:, kp, :, m0 + ci * 128: m0 + (ci + 1) * 128],
                                rhs8[:, 2 * kp:2 * kp + 2, g * bg:(g + 1) * bg],
                                start=(kp == 0), stop=(kp == KP - 1),
                                perf_mode=mybir.MatmulPerfMode.DoubleRow,
                            )
                return
            if merged:
                if first:
                    gates = ((rt[g][:, 0], 0), (rt[g][:, 1], H),
                             (hnt[g][:], 2 * H))
                    wsrc = whhT
                else:
                    gates = ((rt[g][:, 0], 0), (rt[g][:, 1], H),
                             (hnt[g][:], 3 * H), (zit[g][:], 2 * H))
                    wsrc = wT
            elif first:
                # whhT is [r|z|hn]; no in-gate at t=1 (folded into b1n)
                gates = ((rt[g][:], 0), (hnt[g][:], 2 * H), (zit[g][:, 1], H))
                wsrc = whhT
            else:
                gates = ((rt[g][:], 0), (hnt[g][:], 3 * H),
                         (zit[g][:, 0], 2 * H), (zit[g][:, 1], H))
                wsrc = wT
            for dstt, m0 in gates:
                for ci in range(KC):
                    dst = dstt[:, ci, :]
                    for k in range(KC):
                        nc.tensor.matmul(
                            dst, wsrc[:, k, m0 + ci * 128: m0 + (ci + 1) * 128],
                            rhs[:, k, g * bg:(g + 1) * bg],
                            start=(k == 0), stop=(k == KC - 1),
                        )

        def emit_mms_interleaved(t, rhs):
            # same-weight matmuls of all groups adjacent (load-share probe)
            first = (t == 1)
            if first:
                gates = ((rt, 0), (hnt, 2 * H), ([z[:, 1] for z in zit], H))
                wsrc = whhT
            else:
                gates = ((rt, 0), (hnt, 3 * H),
                         ([z[:, 0] for z in zit], 2 * H),
                         ([z[:, 1] for z in zit], H))
                wsrc = wT
            for dstts, m0 in gates:
                for ci in range(KC):
                    for k in range(KC):
                        w_ap = wsrc[:, k, m0 + ci * 128: m0 + (ci + 1) * 128]
                        for g in range(ng):
                            nc.tensor.matmul(
                                dstts[g][:, ci, :], w_ap,
                                rhs[:, k, g * bg:(g + 1) * bg],
                                start=(k == 0), stop=(k == KC - 1),
                            )

        def emit_group_tail(t, g, hbf_prev, hbf_next, hq_next=None):
            """Elementwise chain for group g."""
            first = (t == 1)
            hqnext = (hq_next[:, :, g * bg:(g + 1) * bg]
                      if fp8 else None)
            if merged:
                rzs = ew.tile([128, 2, KC, bg], BF16, tag=f"rzs{g}")
                r2, z2 = rzs[:, 0], rzs[:, 1]
            else:
                r2t = ew.tile([128, KC, bg], BF16, tag=f"r{g}")
                z2t = ew.tile([128, KC, bg], BF16, tag=f"z{g}")
                r2, z2 = r2t[:], z2t[:]
            t1 = ew.tile([128, KC, bg], BF16, tag=f"t1{g}")
            t2t = ew.tile([128, KC, bg], BF16, tag=f"t2{g}")
            t2 = t2t[:]
            n2 = ew.tile([128, KC, bg], BF16, tag=f"n{g}")
            q2 = ew.tile([128, KC, bg], BF16, tag=f"q{g}")
            u2 = ew.tile([128, KC, bg], BF16, tag=f"u{g}")
            v2 = ew.tile([128, KC, bg], BF16, tag=f"v{g}")
            hprev = hbf_prev[:, :, g * bg:(g + 1) * bg]
            hnext = hbf_next[:, :, g * bg:(g + 1) * bg]

            rpre = rt[g][:, 0] if merged else rt[g][:]
            zpre = rt[g][:, 1] if merged else zit[g][:, 1]
            inpre = zit[g][:] if merged else zit[g][:, 0]

            # sigmoids (Act)
            if first or has_rz:
                badd = ew.tile([128, 2, KC, bg], F32, tag=f"badd{g}")
                br = b1r[:] if first else opt["brz"][:, 0]
                bz = b1z[:] if first else opt["brz"][:, 1]
                bsc = FP8_WSCALE if (fp8 and not first) else 1.0
                nc.vector.scalar_tensor_tensor(badd[:, 0], br, bsc, rpre,
                                               OP.mult, OP.add)
                nc.vector.scalar_tensor_tensor(badd[:, 1], bz, bsc, zpre,
                                               OP.mult, OP.add)
                bsc2 = 1.0 / FP8_WSCALE if (fp8 and not first) else 1.0
                if merged:
                    nc.scalar.activation(rzs[:], badd[:], AF.Sigmoid)
                else:
                    nc.scalar.activation(r2, badd[:, 0], AF.Sigmoid,
                                         scale=bsc2)
                    nc.scalar.activation(z2, badd[:, 1], AF.Sigmoid,
                                         scale=bsc2)
            elif merged:
                nc.scalar.activation(rzs[:], rt[g][:], AF.Sigmoid)
            else:
                sc = 1.0 / FP8_WSCALE if fp8 else 1.0
                nc.scalar.activation(r2, rpre, AF.Sigmoid, scale=sc)
                nc.scalar.activation(z2, zpre, AF.Sigmoid, scale=sc)

            # t1 = r * hn, t2 = t1 + in (DVE, on-chain)
            if has_hn:
                hnb = ew.tile([128, KC, bg], F32, tag=f"hnb{g}")
                nc.vector.scalar_tensor_tensor(
                    hnb[:], opt["bhn"][:], FP8_WSCALE if fp8 else 1.0,
                    hnt[g][:], OP.mult, OP.add)
                nc.vector.tensor_mul(t1[:], r2, hnb[:])
            else:
                nc.vector.tensor_mul(t1[:], r2, hnt[g][:])
            if first:
                nc.vector.tensor_add(t2, t1[:], b1n[:])
            else:
                nc.vector.tensor_add(t2, t1[:], inpre)
                if has_in:
                    nc.vector.scalar_tensor_tensor(
                        t2, opt["bin"][:], FP8_WSCALE if fp8 else 1.0,
                        t2, OP.mult, OP.add)

            # off-chain (Pool): q = z*h, u = 1-z
            nc.gpsimd.tensor_mul(q2[:], z2, hprev)
            nc.gpsimd.tensor_scalar(u2[:], z2, -1.0, 1.0, OP.mult, OP.add)

            # TH (Act)
            nc.scalar.activation(n2[:], t2, AF.Tanh,
                                 scale=(1.0 / FP8_WSCALE
                                        if fp8 and not first else 1.0))

            # tail (DVE): v = u*n, h' = v + q (and its fp8 copy for the PE)
            nc.vector.tensor_mul(v2[:], u2[:], n2[:])
            if fp8:
                nc.vector.tensor_add(hqnext, v2[:], q2[:])
            nc.vector.tensor_add(hnext, v2[:], q2[:])

        def proj_mms(hbf):
            pj = psum.tile([Bc, V], F32, tag="proj", bufs=1)
            for k in range(KC):
                nc.tensor.matmul(pj[:], hbf[:, k, :], wproj[:, k, :],
                                 start=(k == 0), stop=(k == KC - 1))
            return pj

        def proj_copy(h_idx, pj):
            # logits slot for h_t is t-1 (outputs are h_1..h_STEPS).
            # GPSIMD cannot access PSUM, so this lives on DVE.
            slot = (h_idx - 1) % out_steps
            if has_proj:
                nc.vector.tensor_add(logits[:, :, slot], pj[:], bproj[:])
            else:
                nc.vector.tensor_copy(logits[:, :, slot], pj[:])

        def emit_body():
            nonlocal hbf_cur, hq_cur
            pj_prev = None
            h_prev_idx = None
            for t in range(1, steps + 1):
                hbf_next = hpool.tile([128, KC, Bc], BF16, tag="hbf", bufs=2)
                hq_next = (hpool.tile([128, KC, Bc], FP8, tag="hq", bufs=2,
                                      name="hq") if fp8 else None)
                if h2:
                    emit_h2_mms(t, hbf_cur)
                else:
                    for g in range(ng):
                        emit_group_mms(t, g, hbf_cur, hq_cur)
                # proj for the previous step's h, after the gate matmuls
                if pj_prev is not None:
                    proj_copy(h_prev_idx, pj_prev)
                pj = proj_mms(hbf_cur) if t > 1 else None
                if h2:
                    emit_h2_tail(t, hbf_cur, hbf_next)
                else:
                    for g in range(ng):
                        emit_group_tail(t, g, hbf_cur, hbf_next, hq_next)
                pj_prev = pj
                h_prev_idx = t - 1
                hbf_cur = hbf_next
                hq_cur = hq_next
            # final projection of h_STEPS
            if pj_prev is not None:
                proj_copy(h_prev_idx, pj_prev)
            pj = proj_mms(hbf_cur)
            proj_copy(steps, pj)

        def emit_body_mm():
            # timing probe: gate matmul streams only, no elementwise/proj
            for t in range(1, steps + 1):
                if mode == "mmi":
                    emit_mms_interleaved(2, hbf_cur)
                else:
                    for g in range(ng):
                        emit_group_mms(max(t, 2), g, hbf_cur, hq_cur)

        if mode in ("mm", "mmi"):
            nc.gpsimd.memset(logits[:], 0.0)
            assert steps % 2 == 0
            with tc.For_i(0, reps):
                emit_body_mm()
        elif mode == "hwloop":
            # timing mode: run the body `reps` times via a hardware loop so
            # the NEFF stays one-body-sized regardless of reps (used by
            # time_harness.py's differential measurement; steps must be even
            # so the double-buffered h tile returns to its initial slot)
            assert steps % 2 == 0
            with tc.For_i(0, reps):
                emit_body()
        else:
            for rep in range(reps):
                emit_body()

        nc.sync.dma_start(out_d[:], logits[:])

    nc.compile()
    return nc


def _prep_inputs(feat, w_hp, b_hp, embed, w_ih, w_hh, b_ih, b_hh, w_proj,
                 b_proj, ngroups=NG):
    f32 = np.float32
    feat = np.asarray(feat, f32)
    w_hp = np.asarray(w_hp, f32)
    b_hp = np.asarray(b_hp, f32)
    embed = np.asarray(embed, f32)
    w_ih = np.asarray(w_ih, f32)
    w_hh = np.asarray(w_hh, f32)
    b_ih = np.asarray(b_ih, f32)
    b_hh = np.asarray(b_hh, f32)
    w_proj = np.asarray(w_proj, f32)
    b_proj = np.asarray(b_proj, f32)

    def bias_full(v):
        # [H] -> [128, KC, Bg]: chunk-major, broadcast over Bg batch cols
        m = v.reshape(KC, 128).T                      # [128, KC]
        return np.ascontiguousarray(
            np.repeat(m[:, :, None], Bc // ngroups, axis=2).astype(f32))

    def chunk_bias(v):          # [H] -> [128, KC] (col c = chunk c)
        return np.ascontiguousarray(v.reshape(KC, 128).T.astype(f32))

    Wc = np.concatenate([
        w_ih[0:H] + w_hh[0:H],
        w_ih[H:2 * H] + w_hh[H:2 * H],
        w_ih[2 * H:3 * H],
        w_hh[2 * H:3 * H],
    ], axis=0)                                   # [4H, H]
    wT = np.ascontiguousarray(Wc.T.reshape(KC, 128, 4 * H).astype(BF16_NP))
    # fp8 DoubleRow layout: [KP, 128, 2, 4H], pair i = k-chunk 2*kp+i,
    # stored x FP8_WSCALE (descaled for free via activation `scale`)
    wf8 = np.ascontiguousarray(
        (Wc.T.reshape(KC, 128, 4 * H)[
            np.arange(KC).reshape(KP, 2)] * FP8_WSCALE
         ).transpose(0, 2, 1, 3).astype(FP8_NP))
    whhT = np.ascontiguousarray(w_hh.T.reshape(KC, 128, 3 * H).astype(BF16_NP))
    whpT = np.ascontiguousarray(w_hp.reshape(KF, 128, H).astype(BF16_NP))
    wproj = np.ascontiguousarray(w_proj.reshape(KC, 128, V).astype(BF16_NP))

    g0 = w_ih @ embed[SOS] + b_ih               # [3H]
    common = dict(wT=wT, wf8=wf8, whhT=whhT, whpT=whpT, wproj=wproj,
                  b1r=bias_full(g0[0:H] + b_hh[0:H]),
                  b1z=bias_full(g0[H:2 * H] + b_hh[H:2 * H]),
                  b1n=bias_full(g0[2 * H:3 * H]))

    biases = set()
    if np.any(b_ih[0:2 * H] + b_hh[0:2 * H]):
        biases.add("rz")
        common["brz"] = np.ascontiguousarray(np.stack(
            [bias_full(b_ih[0:H] + b_hh[0:H]),
             bias_full(b_ih[H:2 * H] + b_hh[H:2 * H])], axis=1))
    if np.any(b_hh[2 * H:]):
        biases.add("hn")
        common["bhn"] = bias_full(b_hh[2 * H:])
    if np.any(b_ih[2 * H:]):
        biases.add("in")
        common["bin"] = bias_full(b_ih[2 * H:])
    if np.any(b_hp):
        biases.add("hp")
        common["bhp"] = chunk_bias(b_hp)
    if np.any(b_proj):
        biases.add("proj")
        common["bproj"] = np.ascontiguousarray(
            np.broadcast_to(b_proj, (Bc, V)).astype(f32))

    featT = feat.T.astype(BF16_NP)               # [FEAT, B]
    in_maps = []
    for c in range(NCORES):
        m = dict(common)
        m["featT"] = np.ascontiguousarray(
            featT[:, c * Bc:(c + 1) * Bc].reshape(KF, 128, Bc))
        in_maps.append(m)
    return frozenset(biases), in_maps


KERNEL_VARIANT = "h2"
KERNEL_NGROUPS = 1


def kernel(**inputs) -> np.ndarray:
    global LAST_RESULTS
    biases, in_maps = _prep_inputs(**inputs, ngroups=KERNEL_NGROUPS)
    key = (biases, KERNEL_VARIANT, KERNEL_NGROUPS)
    if key not in _PROGRAM_CACHE:
        _PROGRAM_CACHE[key] = _build(biases, variant=KERNEL_VARIANT,
                                     ngroups=KERNEL_NGROUPS)
    nc = _PROGRAM_CACHE[key]
    res = run_bass_kernel_spmd(nc, in_maps, list(range(NCORES)))
    LAST_RESULTS = res
    out = np.concatenate([res.results[c]["out"] for c in range(NCORES)], axis=0)
    return np.ascontiguousarray(out)

